# Initial kernel scaffold
#
"""Trainium2 Bass kernel for nn_DenseCoordination (gnn_message_passing).

Math (per batch b):
    hi = s @ W1a ; hj = s @ W1b                       [N, 2D]
    q[i,j,:] = (s_i * s_j) @ W1c + hi_i + hj_j + b1   [N, N, 2D]
    logits[i,j] = relu(q[i,j,:]) @ W2 + b2
    w = softmax(mask(logits), axis=-1) (nan_to_num)
    ctx = w @ s ; gate = ones

Sharding: 8 cores = 4 batches x 2 i-halves. Each core owns b = c//2 and
i in [128*(c%2), 128*(c%2)+128), computes its [128, N] logits / w / ctx.

Device algorithm per core (fixed-j loop, 256 iterations):
    prodT_j[d, i] = sT[d, i-range] * s_j[d]            (DVE tensor_scalar)
    H_j[i, h]     = prodT_j.T @ W1c' (+ hj'_j + b1' via K=1 ones-matmul)
                                                       (PE, fp32r, PSUM)
    logits[:, j]  = sum_{h<npos} relu(H+hi') - sum_{h>=npos} relu(H+hi')
      - DVE variant: relu(H+hi') summed via 2x tensor_tensor_reduce with
        op0=max (relu(a+c) == max(a,-c)+c; the sum of c is folded into the
        reduce's init scalar).
      - ACT variant (every ACT_STRIDE-th j): hi' accumulated on the PE, then
        2x scalar.activation(Relu, accum_out=...).
    W2 is folded into the W1 columns on the host: scale column h of W1* by
    |W2[h]| and permute so positive-sign columns come first (npos of them);
    then logits = sum(relu)|pos - sum(relu)|neg. b2 is dropped (softmax
    shift-invariant; logits are not an output).
Then one masked softmax over [128, 256], w transposed on the PE, and
ctx = w @ s as two fp32 matmuls.
"""

import sys

sys.path.insert(0, "/opt/trn_rl_repo")

import numpy as np

import concourse.bacc as bacc
import concourse.bass as bass
import concourse.tile as tile
from concourse import mybir
from concourse.bass_utils import run_bass_kernel_spmd

D = 256
N = 256
B = 4
H2 = 512  # 2*D
NI = 128  # i rows per core
N_CORES = 8
F32 = mybir.dt.float32
F32R = mybir.dt.float32r
NEG_BIG = -1.0e30
ACT_STRIDE = 3  # every 3rd j uses the scalar-engine epilogue

_BUILD_CACHE: dict = {}


def _build(npos: int, with_loop: bool = True):
    AF = mybir.ActivationFunctionType
    ALU = mybir.AluOpType
    nn = npos
    nneg = H2 - npos

    nc = bacc.Bacc("TRN2", target_bir_lowering=False, debug=False,
                   num_devices=N_CORES)

    s_in = nc.dram_tensor("s", [N, D], F32, kind="ExternalInput").ap()
    sT_in = nc.dram_tensor("sT", [D, N], F32, kind="ExternalInput").ap()
    sTi_in = nc.dram_tensor("sTi", [D, NI], F32, kind="ExternalInput").ap()
    w1a_in = nc.dram_tensor("W1a", [D, H2], F32, kind="ExternalInput").ap()
    w1b_in = nc.dram_tensor("W1b", [D, H2], F32, kind="ExternalInput").ap()
    w1c_in = nc.dram_tensor("W1c", [D, H2], F32, kind="ExternalInput").ap()
    b1_in = nc.dram_tensor("b1", [1, H2], F32, kind="ExternalInput").ap()
    madd_in = nc.dram_tensor("madd", [NI, N], F32, kind="ExternalInput").ap()
    rowind_in = nc.dram_tensor("rowind", [NI, 1], F32, kind="ExternalInput").ap()
    ident_in = nc.dram_tensor("ident", [128, 128], F32, kind="ExternalInput").ap()
    nrep_in = nc.dram_tensor("nrep", [1, 1], mybir.dt.int32,
                             kind="ExternalInput").ap()
    w_out = nc.dram_tensor("w", [NI, N], F32, kind="ExternalOutput").ap()
    ctx_out = nc.dram_tensor("ctx", [NI, D], F32, kind="ExternalOutput").ap()

    with tile.TileContext(nc) as tc:
        with (
            tc.tile_pool(name="persist", bufs=1) as pp,
            tc.tile_pool(name="prod", bufs=6) as prodp,
            tc.tile_pool(name="trash", bufs=3) as trp,
            tc.tile_pool(name="psum", bufs=6, space="PSUM") as psp,
            tc.tile_pool(name="small", bufs=2) as smp,
        ):
            def body(_iv=None):
                # ---- load inputs into SBUF
                s_sb = []
                sT_sb = []
                sTi_sb = []
                w1a_sb = []
                w1b_sb = []
                w1c_sb = []
                for c in range(2):
                    t = pp.tile([128, N], F32, tag=f"s{c}")
                    nc.sync.dma_start(t[:], s_in[128 * c:128 * c + 128, :])
                    s_sb.append(t)
                    t = pp.tile([128, N], F32, tag=f"sT{c}")
                    nc.sync.dma_start(t[:], sT_in[128 * c:128 * c + 128, :])
                    sT_sb.append(t)
                    t = pp.tile([128, NI], F32, tag=f"sTi{c}")
                    nc.sync.dma_start(t[:], sTi_in[128 * c:128 * c + 128, :])
                    sTi_sb.append(t)
                    for nm, src, lst in (("a", w1a_in, w1a_sb),
                                         ("b", w1b_in, w1b_sb),
                                         ("c", w1c_in, w1c_sb)):
                        t = pp.tile([128, H2], F32, tag=f"W1{nm}{c}")
                        nc.sync.dma_start(t[:], src[128 * c:128 * c + 128, :])
                        lst.append(t)
                b1_sb = pp.tile([1, H2], F32, tag="b1")
                nc.sync.dma_start(b1_sb[:], b1_in[:])
                madd_sb = pp.tile([NI, N], F32, tag="madd")
                nc.sync.dma_start(madd_sb[:], madd_in[:])
                rowind_sb = pp.tile([NI, 1], F32, tag="rowind")
                nc.sync.dma_start(rowind_sb[:], rowind_in[:])
                ident_sb = pp.tile([128, 128], F32, tag="ident")
                nc.sync.dma_start(ident_sb[:], ident_in[:])
                ones_sb = pp.tile([128, 128], F32, tag="ones")
                nc.gpsimd.memset(ones_sb[:], 1.0)

                # ---- setup: HJ' = s @ W1b' + b1'  (2 chunks of 128 j's)
                hj_sb = []
                for jc in range(2):
                    ps = psp.tile([128, H2], F32, tag="ps")
                    for kc in range(2):
                        nc.tensor.matmul(
                            ps[:], sT_sb[kc][:, 128 * jc:128 * jc + 128],
                            w1b_sb[kc][:], start=(kc == 0), stop=False)
                    nc.tensor.matmul(ps[:], ones_sb[0:1, :], b1_sb[0:1, :],
                                     start=False, stop=True)
                    t = pp.tile([128, H2], F32, tag=f"hj{jc}")
                    nc.scalar.copy(t[:], ps[:])
                    hj_sb.append(t)

                # ---- setup: HI' = s[i-range] @ W1a' (no b1)
                ps = psp.tile([128, H2], F32, tag="ps")
                for kc in range(2):
                    nc.tensor.matmul(ps[:], sTi_sb[kc][:], w1a_sb[kc][:],
                                     start=(kc == 0), stop=(kc == 1))
                hi_sb = pp.tile([128, H2], F32, tag="hi")
                nc.scalar.copy(hi_sb[:], ps[:])
                neghi_sb = pp.tile([128, H2], F32, tag="neghi")
                nc.vector.tensor_scalar_mul(neghi_sb[:], hi_sb[:], -1.0)
                # init1 = sum(hi'|pos) - sum(hi'|neg)
                hsum_p = smp.tile([128, 1], F32, tag="hsp")
                hsum_n = smp.tile([128, 1], F32, tag="hsn")
                if nn > 0:
                    nc.vector.tensor_reduce(hsum_p[:], hi_sb[:, 0:nn],
                                            axis=mybir.AxisListType.X, op=ALU.add)
                else:
                    nc.gpsimd.memset(hsum_p[:], 0.0)
                if nneg > 0:
                    nc.vector.tensor_reduce(hsum_n[:], hi_sb[:, nn:H2],
                                            axis=mybir.AxisListType.X, op=ALU.add)
                else:
                    nc.gpsimd.memset(hsum_n[:], 0.0)
                init1 = pp.tile([128, 1], F32, tag="init1")
                nc.vector.tensor_tensor(init1[:], hsum_p[:], hsum_n[:],
                                        op=ALU.subtract)

                # ---- main fixed-j loop
                logits = pp.tile([NI, N], F32, tag="logits")
                tmpacc = pp.tile([NI, N], F32, tag="tmpacc")
                n_act = len([j for j in range(N)
                             if j % ACT_STRIDE == ACT_STRIDE - 1])
                accp = pp.tile([NI, max(n_act, 1)], F32, tag="accp")
                accn = pp.tile([NI, max(n_act, 1)], F32, tag="accn")
                ja = 0
                for j in range(N):
                    jc, jr = j // 128, j % 128
                    pt = []
                    for kc in range(2):
                        t = prodp.tile([128, NI], F32, tag=f"pt{kc}")
                        nc.vector.tensor_scalar_mul(
                            t[:], sTi_sb[kc][:],
                            sT_sb[kc][:, j:j + 1])
                        pt.append(t)
                    ps = psp.tile([128, H2], F32, tag="ps")
                    is_act = (j % ACT_STRIDE == ACT_STRIDE - 1)
                    for kc in range(2):
                        nc.tensor.matmul(
                            ps[:], pt[kc][:].bitcast(F32R),
                            w1c_sb[kc][:].bitcast(F32R),
                            start=(kc == 0), stop=False)
                    nc.tensor.matmul(
                        ps[:], ones_sb[jr:jr + 1, :].bitcast(F32R),
                        hj_sb[jc][jr:jr + 1, :].bitcast(F32R),
                        start=False, stop=not is_act)
                    if not is_act:
                        # DVE epilogue: logits[:,j] = init1
                        #   + sum_pos max(H, -hi') - sum_neg max(H, -hi')
                        if nn > 0:
                            tr1 = trp.tile([128, nn], F32, tag="tr1")
                            nc.vector.tensor_tensor_reduce(
                                out=tr1[:], in0=ps[:, 0:nn],
                                in1=neghi_sb[:, 0:nn], scale=1.0,
                                scalar=init1[:, 0:1], op0=ALU.max, op1=ALU.add,
                                accum_out=tmpacc[:, j:j + 1])
                            src2 = tmpacc[:, j:j + 1]
                        else:
                            src2 = init1[:, 0:1]
                        if nneg > 0:
                            tr2 = trp.tile([128, nneg], F32, tag="tr2")
                            nc.vector.tensor_tensor_reduce(
                                out=tr2[:], in0=ps[:, nn:H2],
                                in1=neghi_sb[:, nn:H2], scale=-1.0,
                                scalar=src2, op0=ALU.max, op1=ALU.add,
                                accum_out=logits[:, j:j + 1])
                        else:
                            nc.vector.tensor_copy(logits[:, j:j + 1], src2)
                    else:
                        # ACT epilogue: add hi' on the PE, then relu+accum
                        for kc in range(2):
                            nc.tensor.matmul(
                                ps[:], sTi_sb[kc][:].bitcast(F32R),
                                w1a_sb[kc][:].bitcast(F32R),
                                start=False, stop=(kc == 1))
                        if nn > 0:
                            tr1 = trp.tile([128, nn], F32, tag="tr1")
                            nc.scalar.activation(tr1[:], ps[:, 0:nn], AF.Relu,
                                                 accum_out=accp[:, ja:ja + 1])
                        else:
                            nc.gpsimd.memset(accp[:, ja:ja + 1], 0.0)
                        if nneg > 0:
                            tr2 = trp.tile([128, nneg], F32, tag="tr2")
                            nc.scalar.activation(tr2[:], ps[:, nn:H2], AF.Relu,
                                                 accum_out=accn[:, ja:ja + 1])
                        else:
                            nc.gpsimd.memset(accn[:, ja:ja + 1], 0.0)
                        ja += 1
                if n_act > 0:
                    # logits[:, ACT_STRIDE-1::ACT_STRIDE] = accp - accn
                    nc.vector.tensor_tensor(
                        logits[:, ACT_STRIDE - 1::ACT_STRIDE][:, 0:n_act],
                        accp[:, 0:n_act], accn[:, 0:n_act], op=ALU.subtract)

                # ---- masked softmax over j
                l2 = pp.tile([NI, N], F32, tag="l2")
                nc.vector.tensor_tensor(l2[:], logits[:], madd_sb[:], op=ALU.add)
                negm = smp.tile([NI, 1], F32, tag="negm")
                nc.vector.tensor_reduce(negm[:], l2[:],
                                        axis=mybir.AxisListType.X, op=ALU.max,
                                        negate=True)
                ex = pp.tile([NI, N], F32, tag="ex")
                ssum = smp.tile([NI, 1], F32, tag="ssum")
                nc.scalar.activation(ex[:], l2[:], AF.Exp, bias=negm[:, 0:1],
                                     accum_out=ssum[:, 0:1])
                rec = smp.tile([NI, 1], F32, tag="rec")
                nc.vector.reciprocal(rec[:], ssum[:])
                rec2 = smp.tile([NI, 1], F32, tag="rec2")
                nc.vector.tensor_tensor(rec2[:], rec[:], rowind_sb[:],
                                        op=ALU.mult)
                w_sb = pp.tile([NI, N], F32, tag="wsb")
                nc.vector.tensor_scalar_mul(w_sb[:], ex[:], rec2[:, 0:1])

                # ---- ctx = w @ s  (transpose w on the PE first)
                wt_sb = []
                for jc in range(2):
                    pst = psp.tile([128, 128], F32, tag="pst")
                    nc.tensor.transpose(pst[:], w_sb[:, 128 * jc:128 * jc + 128],
                                        ident_sb[:])
                    t = smp.tile([128, 128], F32, tag=f"wt{jc}")
                    nc.vector.tensor_copy(t[:], pst[:])
                    wt_sb.append(t)
                psc = psp.tile([128, D], F32, tag="psc")
                for jc in range(2):
                    nc.tensor.matmul(psc[:], wt_sb[jc][:], s_sb[jc][:],
                                     start=(jc == 0), stop=(jc == 1))
                ctx_sb = pp.tile([NI, D], F32, tag="ctxsb")
                nc.scalar.copy(ctx_sb[:], psc[:])

                # ---- outputs
                nc.sync.dma_start(w_out[:], w_sb[:])
                nc.sync.dma_start(ctx_out[:], ctx_sb[:])

            if with_loop:
                nrep_sb = pp.tile([1, 1], mybir.dt.int32, tag="nrep")
                nc.sync.dma_start(nrep_sb[:], nrep_in[:])
                rv = nc.values_load(nrep_sb[0:1, 0:1], min_val=1,
                                    max_val=100000,
                                    skip_runtime_bounds_check=True)
                with tc.For_i(0, rv, 1):
                    body()
            else:
                body()

    nc.compile()
    return nc


def _prep(s, W1, b1, W2, b2, adj_allowed, active_mask, act_mask):
    s = np.ascontiguousarray(np.asarray(s, dtype=np.float32))
    W1 = np.asarray(W1, dtype=np.float32)
    b1 = np.asarray(b1, dtype=np.float32).reshape(-1)
    W2 = np.asarray(W2, dtype=np.float32).reshape(-1)  # [2D]
    adj = np.asarray(adj_allowed)
    am = np.asarray(active_mask)
    km = np.asarray(act_mask)

    pos = W2 >= 0.0
    perm = np.concatenate([np.nonzero(pos)[0], np.nonzero(~pos)[0]])
    npos = int(pos.sum())
    w2p = np.abs(W2[perm])
    W1a = np.ascontiguousarray(W1[:D][:, perm] * w2p[None, :])
    W1b = np.ascontiguousarray(W1[D:2 * D][:, perm] * w2p[None, :])
    W1c = np.ascontiguousarray(W1[2 * D:][:, perm] * w2p[None, :])
    b1p = np.ascontiguousarray((b1[perm] * w2p)[None, :])

    valid = (adj > 0) & (am > 0)[:, None, :] & (km > 0)[:, :, None]
    madd = np.where(valid, np.float32(0.0), np.float32(NEG_BIG))
    rowind = valid.any(axis=-1).astype(np.float32)
    ident = np.eye(128, dtype=np.float32)
    return s, W1a, W1b, W1c, b1p, madd, rowind, ident, npos


def _in_maps(s, W1a, W1b, W1c, b1p, madd, rowind, ident, nrep):
    nrep_arr = np.full((1, 1), nrep, dtype=np.int32)
    maps = []
    for c in range(N_CORES):
        b, i0 = c // 2, NI * (c % 2)
        sb = s[b]
        sTb = np.ascontiguousarray(sb.T)
        maps.append({
            "s": sb,
            "sT": sTb,
            "sTi": np.ascontiguousarray(sTb[:, i0:i0 + NI]),
            "W1a": W1a, "W1b": W1b, "W1c": W1c, "b1": b1p,
            "madd": np.ascontiguousarray(madd[b, i0:i0 + NI]),
            "rowind": np.ascontiguousarray(rowind[b, i0:i0 + NI, None]),
            "ident": ident,
            "nrep": nrep_arr,
        })
    return maps


def _gather(results):
    w = np.empty((B, N, N), dtype=np.float32)
    ctx = np.empty((B, N, D), dtype=np.float32)
    for c in range(N_CORES):
        b, i0 = c // 2, NI * (c % 2)
        w[b, i0:i0 + NI] = results[c]["w"]
        ctx[b, i0:i0 + NI] = results[c]["ctx"]
    gate = np.ones((B, N, N), dtype=np.float32)
    return ctx, gate, w


def _get_program(npos, with_loop=True):
    key = (npos, with_loop)
    if key not in _BUILD_CACHE:
        _BUILD_CACHE[key] = _build(npos, with_loop=with_loop)
    return _BUILD_CACHE[key]


def run(nrep, *, with_loop=True, **inputs):
    """Run the device kernel with the compute body repeated `nrep` times."""
    s, W1a, W1b, W1c, b1p, madd, rowind, ident, npos = _prep(**inputs)
    nc = _get_program(npos, with_loop=with_loop)
    maps = _in_maps(s, W1a, W1b, W1c, b1p, madd, rowind, ident, nrep)
    res = run_bass_kernel_spmd(nc, maps, list(range(N_CORES)))
    return _gather(res.results)


def kernel(**inputs):
    return run(1, **inputs)


# revision 6
# speedup vs baseline: 1.0013x; 1.0013x over previous
"""Trainium2 Bass kernel for nn_DenseCoordination (gnn_message_passing).

Math (per batch b):
    hi = s @ W1a ; hj = s @ W1b                       [N, 2D]
    q[i,j,:] = (s_i * s_j) @ W1c + hi_i + hj_j + b1   [N, N, 2D]
    logits[i,j] = relu(q[i,j,:]) @ W2 + b2
    w = softmax(mask(logits), axis=-1) (nan_to_num)
    ctx = w @ s ; gate = ones

Sharding: 8 cores = 4 batches x 2 i-halves. Each core owns b = c//2 and
i in [128*(c%2), 128*(c%2)+128), computes its [128, N] logits / w / ctx.

Device algorithm per core (fixed-j loop, 256 iterations):
    prodT_j[d, i] = sT[d, i-range] * s_j[d]            (DVE tensor_scalar)
    H_j[i, h]     = prodT_j.T @ W1c' (+ hj'_j + b1' via K=1 ones-matmul)
                                                       (PE, fp32r, PSUM)
    logits[:, j]  = sum_{h<npos} relu(H+hi') - sum_{h>=npos} relu(H+hi')
      - DVE variant: relu(H+hi') summed via 2x tensor_tensor_reduce with
        op0=max (relu(a+c) == max(a,-c)+c; the sum of c is folded into the
        reduce's init scalar).
      - ACT variant (every ACT_STRIDE-th j): hi' accumulated on the PE, then
        2x scalar.activation(Relu, accum_out=...).
    W2 is folded into the W1 columns on the host: scale column h of W1* by
    |W2[h]| and permute so positive-sign columns come first (npos of them);
    then logits = sum(relu)|pos - sum(relu)|neg. b2 is dropped (softmax
    shift-invariant; logits are not an output).
Then one masked softmax over [128, 256], w transposed on the PE, and
ctx = w @ s as two fp32 matmuls.
"""

import sys

sys.path.insert(0, "/opt/trn_rl_repo")

import numpy as np

import concourse.bacc as bacc
import concourse.bass as bass
import concourse.tile as tile
from concourse import mybir
from concourse.bass_utils import run_bass_kernel_spmd

D = 256
N = 256
B = 4
H2 = 512  # 2*D
NI = 128  # i rows per core
N_CORES = 8
F32 = mybir.dt.float32
F32R = mybir.dt.float32r
NEG_BIG = -1.0e30
ACT_STRIDE = 3  # every 3rd j uses the scalar-engine epilogue

_BUILD_CACHE: dict = {}


def _build(npos: int, with_loop: bool = True):
    AF = mybir.ActivationFunctionType
    ALU = mybir.AluOpType
    nn = npos
    nneg = H2 - npos

    nc = bacc.Bacc("TRN2", target_bir_lowering=False, debug=False,
                   num_devices=N_CORES)

    s_in = nc.dram_tensor("s", [N, D], F32, kind="ExternalInput").ap()
    sT_in = nc.dram_tensor("sT", [D, N], F32, kind="ExternalInput").ap()
    sTi_in = nc.dram_tensor("sTi", [D, NI], F32, kind="ExternalInput").ap()
    w1a_in = nc.dram_tensor("W1a", [D, H2], F32, kind="ExternalInput").ap()
    w1b_in = nc.dram_tensor("W1b", [D, H2], F32, kind="ExternalInput").ap()
    w1c_in = nc.dram_tensor("W1c", [D, H2], F32, kind="ExternalInput").ap()
    b1_in = nc.dram_tensor("b1", [1, H2], F32, kind="ExternalInput").ap()
    madd_in = nc.dram_tensor("madd", [NI, N], F32, kind="ExternalInput").ap()
    rowind_in = nc.dram_tensor("rowind", [NI, 1], F32, kind="ExternalInput").ap()
    ident_in = nc.dram_tensor("ident", [128, 128], F32, kind="ExternalInput").ap()
    nrep_in = nc.dram_tensor("nrep", [1, 1], mybir.dt.int32,
                             kind="ExternalInput").ap()
    w_out = nc.dram_tensor("w", [NI, N], F32, kind="ExternalOutput").ap()
    ctx_out = nc.dram_tensor("ctx", [NI, D], F32, kind="ExternalOutput").ap()

    with tile.TileContext(nc) as tc:
        with (
            tc.tile_pool(name="persist", bufs=1) as pp,
            tc.tile_pool(name="prod", bufs=6) as prodp,
            tc.tile_pool(name="trash", bufs=3) as trp,
            tc.tile_pool(name="psum", bufs=5, space="PSUM") as psp,
            tc.tile_pool(name="psum2", bufs=1, space="PSUM") as psp2,
            tc.tile_pool(name="small", bufs=2) as smp,
        ):
            def body(_iv=None):
                # ---- load inputs into SBUF
                s_sb = []
                sT_sb = []
                sTi_sb = []
                w1a_sb = []
                w1b_sb = []
                w1c_sb = []
                for c in range(2):
                    t = pp.tile([128, N], F32, tag=f"s{c}")
                    nc.sync.dma_start(t[:], s_in[128 * c:128 * c + 128, :])
                    s_sb.append(t)
                    t = pp.tile([128, N], F32, tag=f"sT{c}")
                    nc.sync.dma_start(t[:], sT_in[128 * c:128 * c + 128, :])
                    sT_sb.append(t)
                    t = pp.tile([128, NI], F32, tag=f"sTi{c}")
                    nc.sync.dma_start(t[:], sTi_in[128 * c:128 * c + 128, :])
                    sTi_sb.append(t)
                    for nm, src, lst in (("a", w1a_in, w1a_sb),
                                         ("b", w1b_in, w1b_sb),
                                         ("c", w1c_in, w1c_sb)):
                        t = pp.tile([128, H2], F32, tag=f"W1{nm}{c}")
                        nc.sync.dma_start(t[:], src[128 * c:128 * c + 128, :])
                        lst.append(t)
                b1_sb = pp.tile([1, H2], F32, tag="b1")
                nc.sync.dma_start(b1_sb[:], b1_in[:])
                madd_sb = pp.tile([NI, N], F32, tag="madd")
                nc.sync.dma_start(madd_sb[:], madd_in[:])
                rowind_sb = pp.tile([NI, 1], F32, tag="rowind")
                nc.sync.dma_start(rowind_sb[:], rowind_in[:])
                ident_sb = pp.tile([128, 128], F32, tag="ident")
                nc.sync.dma_start(ident_sb[:], ident_in[:])
                ones_sb = pp.tile([128, 128], F32, tag="ones")
                nc.gpsimd.memset(ones_sb[:], 1.0)

                # fp32r-rounded copies for the hot-loop matmul operands
                w1c_r = []
                w1a_r = []
                sTi_r = []
                for c in range(2):
                    t = pp.tile([128, H2], F32R, tag=f"W1cr{c}")
                    nc.vector.tensor_copy(t[:], w1c_sb[c][:])
                    w1c_r.append(t)
                    t = pp.tile([128, H2], F32R, tag=f"W1ar{c}")
                    nc.vector.tensor_copy(t[:], w1a_sb[c][:])
                    w1a_r.append(t)
                    t = pp.tile([128, NI], F32R, tag=f"sTir{c}")
                    nc.vector.tensor_copy(t[:], sTi_sb[c][:])
                    sTi_r.append(t)
                ident_r = pp.tile([128, 128], F32R, tag="identr")
                nc.vector.tensor_copy(ident_r[:], ident_sb[:])

                # ---- setup: HJ' = s @ W1b' + b1'  (2 chunks of 128 j's)
                hj_sb = []
                for jc in range(2):
                    ps = psp.tile([128, H2], F32, tag="ps")
                    for kc in range(2):
                        nc.tensor.matmul(
                            ps[:], sT_sb[kc][:, 128 * jc:128 * jc + 128],
                            w1b_sb[kc][:], start=(kc == 0), stop=False)
                    nc.tensor.matmul(ps[:], ones_sb[0:1, :], b1_sb[0:1, :],
                                     start=False, stop=True)
                    t = pp.tile([128, H2], F32R, tag=f"hj{jc}")
                    nc.scalar.copy(t[:], ps[:])
                    hj_sb.append(t)

                # ---- setup: HI' = s[i-range] @ W1a' (no b1)
                ps = psp.tile([128, H2], F32, tag="ps")
                for kc in range(2):
                    nc.tensor.matmul(ps[:], sTi_sb[kc][:], w1a_sb[kc][:],
                                     start=(kc == 0), stop=(kc == 1))
                hi_sb = pp.tile([128, H2], F32, tag="hi")
                nc.scalar.copy(hi_sb[:], ps[:])
                neghi_sb = pp.tile([128, H2], F32, tag="neghi")
                nc.vector.tensor_scalar_mul(neghi_sb[:], hi_sb[:], -1.0)
                # init1 = sum(hi'|pos) - sum(hi'|neg)
                hsum_p = smp.tile([128, 1], F32, tag="hsp")
                hsum_n = smp.tile([128, 1], F32, tag="hsn")
                if nn > 0:
                    nc.vector.tensor_reduce(hsum_p[:], hi_sb[:, 0:nn],
                                            axis=mybir.AxisListType.X, op=ALU.add)
                else:
                    nc.gpsimd.memset(hsum_p[:], 0.0)
                if nneg > 0:
                    nc.vector.tensor_reduce(hsum_n[:], hi_sb[:, nn:H2],
                                            axis=mybir.AxisListType.X, op=ALU.add)
                else:
                    nc.gpsimd.memset(hsum_n[:], 0.0)
                init1 = pp.tile([128, 1], F32, tag="init1")
                nc.vector.tensor_sub(init1[:], hsum_p[:], hsum_n[:])

                # ---- main fixed-j loop
                logits = pp.tile([NI, N], F32, tag="logits")
                n_act = len([j for j in range(N)
                             if j % ACT_STRIDE == ACT_STRIDE - 1])
                accp = pp.tile([NI, N], F32, tag="accp")
                accn = pp.tile([NI, N], F32, tag="accn")
                for j in range(N):
                    jc, jr = j // 128, j % 128
                    pt = []
                    for kc in range(2):
                        t = prodp.tile([128, NI], F32R, tag=f"pt{kc}")
                        nc.vector.tensor_scalar_mul(
                            t[:], sTi_sb[kc][:],
                            sT_sb[kc][:, j:j + 1])
                        pt.append(t)
                    ps = psp.tile([128, H2], F32, tag="ps")
                    is_act = (j % ACT_STRIDE == ACT_STRIDE - 1)
                    for kc in range(2):
                        nc.tensor.matmul(
                            ps[:], pt[kc][:], w1c_r[kc][:],
                            start=(kc == 0), stop=False)
                    # += hj'_j + b1': lhsT[k, m] = ident[k, jr] (one-hot row
                    # jr, broadcast along the free dim), rhs = full HJ chunk.
                    nc.tensor.matmul(
                        ps[:],
                        ident_r[:, jr:jr + 1].to_broadcast((128, 128)),
                        hj_sb[jc][:],
                        start=False, stop=not is_act)
                    if not is_act:
                        # DVE epilogue: accp/accn[:, j] = sum_range max(H, -hi')
                        # (relu(H+hi) = max(H,-hi)+hi; the hi sums enter via
                        # init1 after the loop)
                        if nn > 0:
                            tr1 = trp.tile([128, nn], F32, tag="tr1")
                            nc.vector.scalar_tensor_tensor(
                                out=tr1[:], in0=ps[:, 0:nn], scalar=0.0,
                                in1=neghi_sb[:, 0:nn], op0=ALU.add,
                                op1=ALU.max, accum_out=accp[:, j:j + 1])
                        else:
                            nc.gpsimd.memset(accp[:, j:j + 1], 0.0)
                        if nneg > 0:
                            tr2 = trp.tile([128, nneg], F32, tag="tr2")
                            nc.vector.scalar_tensor_tensor(
                                out=tr2[:], in0=ps[:, nn:H2], scalar=0.0,
                                in1=neghi_sb[:, nn:H2], op0=ALU.add,
                                op1=ALU.max, accum_out=accn[:, j:j + 1])
                        else:
                            nc.gpsimd.memset(accn[:, j:j + 1], 0.0)
                    else:
                        # ACT epilogue: add hi' on the PE, then relu+accum
                        for kc in range(2):
                            nc.tensor.matmul(
                                ps[:], sTi_r[kc][:], w1a_r[kc][:],
                                start=False, stop=(kc == 1))
                        if nn > 0:
                            tr1 = trp.tile([128, nn], F32, tag="tr1")
                            nc.scalar.activation(tr1[:], ps[:, 0:nn], AF.Relu,
                                                 accum_out=accp[:, j:j + 1])
                        else:
                            nc.gpsimd.memset(accp[:, j:j + 1], 0.0)
                        if nneg > 0:
                            tr2 = trp.tile([128, nneg], F32, tag="tr2")
                            nc.scalar.activation(tr2[:], ps[:, nn:H2], AF.Relu,
                                                 accum_out=accn[:, j:j + 1])
                        else:
                            nc.gpsimd.memset(accn[:, j:j + 1], 0.0)
                # logits = (accp - accn) + init1; ACT-variant columns
                # already include hi' inside the relu, so un-add init1 there.
                lt1 = pp.tile([NI, N], F32, tag="lt1")
                nc.vector.tensor_sub(lt1[:], accp[:], accn[:])
                nc.vector.tensor_scalar_add(logits[:], lt1[:], init1[:, 0:1])
                if n_act > 0:
                    av = logits[:, ACT_STRIDE - 1::ACT_STRIDE][:, 0:n_act]
                    nc.vector.tensor_scalar_sub(av, av, init1[:, 0:1])

                # ---- masked softmax over j
                l2 = pp.tile([NI, N], F32, tag="l2")
                nc.vector.tensor_add(l2[:], logits[:], madd_sb[:])
                negm = smp.tile([NI, 1], F32, tag="negm")
                nc.vector.tensor_reduce(negm[:], l2[:],
                                        axis=mybir.AxisListType.X, op=ALU.max,
                                        negate=True)
                ex = pp.tile([NI, N], F32, tag="ex")
                ssum = smp.tile([NI, 1], F32, tag="ssum")
                nc.scalar.activation(ex[:], l2[:], AF.Exp, bias=negm[:, 0:1],
                                     accum_out=ssum[:, 0:1])
                rec = smp.tile([NI, 1], F32, tag="rec")
                nc.vector.reciprocal(rec[:], ssum[:])
                rec2 = smp.tile([NI, 1], F32, tag="rec2")
                nc.vector.tensor_mul(rec2[:], rec[:], rowind_sb[:])
                w_sb = pp.tile([NI, N], F32, tag="wsb")
                nc.vector.tensor_scalar_mul(w_sb[:], ex[:], rec2[:, 0:1])

                # ---- ctx = w @ s  (transpose w on the PE first)
                wt_sb = []
                for jc in range(2):
                    pst = psp2.tile([128, 128], F32, tag="pst")
                    nc.tensor.transpose(pst[:], w_sb[:, 128 * jc:128 * jc + 128],
                                        ident_sb[:])
                    t = smp.tile([128, 128], F32, tag=f"wt{jc}")
                    nc.vector.tensor_copy(t[:], pst[:])
                    wt_sb.append(t)
                psc = psp2.tile([128, D], F32, tag="psc")
                for jc in range(2):
                    nc.tensor.matmul(psc[:], wt_sb[jc][:], s_sb[jc][:],
                                     start=(jc == 0), stop=(jc == 1))
                ctx_sb = pp.tile([NI, D], F32, tag="ctxsb")
                nc.scalar.copy(ctx_sb[:], psc[:])

                # ---- outputs
                nc.sync.dma_start(w_out[:], w_sb[:])
                nc.sync.dma_start(ctx_out[:], ctx_sb[:])

            if with_loop:
                nrep_sb = pp.tile([1, 1], mybir.dt.int32, tag="nrep")
                nc.sync.dma_start(nrep_sb[:], nrep_in[:])
                rv = nc.values_load(nrep_sb[0:1, 0:1], min_val=1,
                                    max_val=100000,
                                    skip_runtime_bounds_check=True)
                with tc.For_i(0, rv, 1):
                    body()
            else:
                body()

    nc.compile()
    return nc


def _prep(s, W1, b1, W2, b2, adj_allowed, active_mask, act_mask):
    s = np.ascontiguousarray(np.asarray(s, dtype=np.float32))
    W1 = np.asarray(W1, dtype=np.float32)
    b1 = np.asarray(b1, dtype=np.float32).reshape(-1)
    W2 = np.asarray(W2, dtype=np.float32).reshape(-1)  # [2D]
    adj = np.asarray(adj_allowed)
    am = np.asarray(active_mask)
    km = np.asarray(act_mask)

    pos = W2 >= 0.0
    perm = np.concatenate([np.nonzero(pos)[0], np.nonzero(~pos)[0]])
    npos = int(pos.sum())
    w2p = np.abs(W2[perm])
    W1a = np.ascontiguousarray(W1[:D][:, perm] * w2p[None, :])
    W1b = np.ascontiguousarray(W1[D:2 * D][:, perm] * w2p[None, :])
    W1c = np.ascontiguousarray(W1[2 * D:][:, perm] * w2p[None, :])
    b1p = np.ascontiguousarray((b1[perm] * w2p)[None, :])

    valid = (adj > 0) & (am > 0)[:, None, :] & (km > 0)[:, :, None]
    madd = np.where(valid, np.float32(0.0), np.float32(NEG_BIG))
    rowind = valid.any(axis=-1).astype(np.float32)
    ident = np.eye(128, dtype=np.float32)
    return s, W1a, W1b, W1c, b1p, madd, rowind, ident, npos


def _in_maps(s, W1a, W1b, W1c, b1p, madd, rowind, ident, nrep):
    nrep_arr = np.full((1, 1), nrep, dtype=np.int32)
    maps = []
    for c in range(N_CORES):
        b, i0 = c // 2, NI * (c % 2)
        sb = s[b]
        sTb = np.ascontiguousarray(sb.T)
        maps.append({
            "s": sb,
            "sT": sTb,
            "sTi": np.ascontiguousarray(sTb[:, i0:i0 + NI]),
            "W1a": W1a, "W1b": W1b, "W1c": W1c, "b1": b1p,
            "madd": np.ascontiguousarray(madd[b, i0:i0 + NI]),
            "rowind": np.ascontiguousarray(rowind[b, i0:i0 + NI, None]),
            "ident": ident,
            "nrep": nrep_arr,
        })
    return maps


def _gather(results):
    w = np.empty((B, N, N), dtype=np.float32)
    ctx = np.empty((B, N, D), dtype=np.float32)
    for c in range(N_CORES):
        b, i0 = c // 2, NI * (c % 2)
        w[b, i0:i0 + NI] = results[c]["w"]
        ctx[b, i0:i0 + NI] = results[c]["ctx"]
    gate = np.ones((B, N, N), dtype=np.float32)
    return ctx, gate, w


def _get_program(npos, with_loop=True):
    key = (npos, with_loop)
    if key not in _BUILD_CACHE:
        _BUILD_CACHE[key] = _build(npos, with_loop=with_loop)
    return _BUILD_CACHE[key]


def run(nrep, *, with_loop=True, **inputs):
    """Run the device kernel with the compute body repeated `nrep` times."""
    s, W1a, W1b, W1c, b1p, madd, rowind, ident, npos = _prep(**inputs)
    nc = _get_program(npos, with_loop=with_loop)
    maps = _in_maps(s, W1a, W1b, W1c, b1p, madd, rowind, ident, nrep)
    res = run_bass_kernel_spmd(nc, maps, list(range(N_CORES)))
    return _gather(res.results)


def kernel(**inputs):
    return run(1, **inputs)
